# revision 1
# baseline (speedup 1.0000x reference)
"""Trainium2 Bass kernel for a bidirectional linear-attention transformer layer.

Contract: kernel(**inputs) takes the FULL unsharded inputs (as produced by the
problem's setup_inputs()) and returns the FULL (B, T, D) float32 output.

Sharding: token-parallel across 8 NeuronCores. Core c handles batch c//4,
query tokens [ (c%4)*512, (c%4)*512+512 ), with a +-W token halo (W=128 for
decay=0.9). The decay mask d^|i-j| factorizes per 256-token query chunk into
  - a Q-side scale d^{+di} (past/diag) or d^{-di} (future), baked into two
    pre-scaled copies of Q,
  - a K-side scale folded into V rows and into an extra "ones" column of V
    that simultaneously produces the normalization denominator z,
  - an elementwise 256x256 mask for the diagonal zone only.
All heavy matmuls run in fp32r (fp32 with 11-bit mantissa, 1 cycle/row).
"""

import math
import os
from contextlib import ExitStack

import numpy as np

import concourse.bacc as bacc
import concourse.mybir as mybir
import concourse.tile as tile
from concourse import bass_utils
from concourse.alu_op_type import AluOpType

F32 = mybir.dt.float32
F32R = mybir.dt.float32r
AF = mybir.ActivationFunctionType

B, T, D, H, FF = 2, 2048, 512, 8, 2048
HD = D // H          # 64
NCORES = 8
SH = 4               # token shards per batch
TPC = T // SH        # 512 query tokens per core
CH = 256             # query chunk width
NQC = TPC // CH      # 2 query chunks per core
KC = D // 128        # 4 contraction chunks over D
FFC = FF // 128      # 16 ff chunks


def _np_fp32r(v):
    """Round fp32 to fp32r (RNE at mantissa bit 12) -- matches walrus."""
    u = np.ascontiguousarray(np.asarray(v, np.float32)).view(np.uint32)
    keep = u & np.uint32(0xFFFFF000)
    rem = u & np.uint32(0x00000FFF)
    lsb = (u >> np.uint32(12)) & np.uint32(1)
    ru = (rem > 0x800) | ((rem == 0x800) & (lsb == 1))
    return (keep + (ru.astype(np.uint32) << np.uint32(12))).view(np.float32)


# ---------------------------------------------------------------------------
# device kernel build (depends only on W)
# ---------------------------------------------------------------------------

_BUILD_CACHE = {}


def _build(W, uniform, nobias):
    nW = W // 128            # past/future 128-blocks per q-chunk
    L = TPC + 2 * W          # local tokens per core
    NT = L // 128            # 128-token tiles per core
    NQB = TPC // 128         # 4 query 128-blocks
    ND = 2 * nW + 2          # key blocks per q-chunk (past nW, diag 2, fut nW)
    NHM = 1 if uniform else H    # distinct decay tables needed
    NQP = 1 if uniform else KC
    # nobias: b1@wv, bo, bf2 are all zero -> skip the K=1 bias-row matmuls

    nc = bacc.Bacc("TRN2", target_bir_lowering=False, debug=False)

    din = {}

    def dram_in(name, shape, dt=F32):
        din[name] = nc.dram_tensor(name, shape, dt, kind="ExternalInput").ap()
        return din[name]

    xsl = dram_in("xsl", [L, D])                     # x slice (zero-padded)
    wq = dram_in("wq", [D, D], F32R)                 # g1-folded
    wk = dram_in("wk", [D, D], F32R)
    wv = dram_in("wv", [D, D], F32R)
    wo = dram_in("wo", [D, D], F32R)
    w1 = dram_in("w1", [D, FF], F32R)                # g2-folded
    w2 = dram_in("w2", [FF, D], F32R)
    bq = dram_in("bq", [D])                          # b1 @ wq
    bk = dram_in("bk", [D])
    bvrow = dram_in("bvrow", [1, D], F32R)           # b1 @ wv
    borow = dram_in("borow", [1, D], F32R)           # bo
    bf1 = dram_in("bf1", [FF])                       # b2 @ w1 + bf1
    bf2row = dram_in("bf2row", [1, D], F32R)
    # decay-derived, host computed per core:
    # layout: [128, NT*H], element (p, i*H+h) = scale for token i*128+p, head h
    vneg = dram_in("vneg", [128, (L // 128) * H])
    vpos = dram_in("vpos", [128, (L // 128) * H])
    vpln = dram_in("vpln", [128, (L // 128) * H])
    qposb = dram_in("qposb", [NQP, 128, CH])         # per head-pair bcast
    qnegb = dram_in("qnegb", [NQP, 128, CH])
    dmask = dram_in("dmask", [NHM, 2, 128, CH])      # diagonal-zone mask
    ident = dram_in("ident", [128, 128])

    out = nc.dram_tensor("out", [TPC, D], F32, kind="ExternalOutput").ap()

    with tile.TileContext(nc) as tc, ExitStack() as top:
        cpool = top.enter_context(tc.tile_pool(name="const", bufs=1))
        xpool = top.enter_context(tc.tile_pool(name="x", bufs=1))
        x2pool = top.enter_context(tc.tile_pool(name="x2", bufs=1))

        # x tiles first: LN1 + transposes are the critical path at startup
        x_t = [xpool.tile([128, D], F32, tag=f"x{i}", name=f"x{i}") for i in range(NT)]
        for i in range(NT):
            nc.sync.dma_start(x_t[i][:], xsl[128 * i:128 * (i + 1), :])
        # ---- constants -------------------------------------------------
        id_t = cpool.tile([128, 128], F32)
        nc.sync.dma_start(id_t[:], ident[:])
        bq_t = cpool.tile([128, KC], F32)
        nc.sync.dma_start(bq_t[:], bq.rearrange("(c p) -> p c", p=128))
        bk_t = cpool.tile([128, KC], F32)
        nc.sync.dma_start(bk_t[:], bk.rearrange("(c p) -> p c", p=128))
        bf1_t = cpool.tile([128, FFC], F32)
        nc.sync.dma_start(bf1_t[:], bf1.rearrange("(c p) -> p c", p=128))
        bvrow_t = cpool.tile([1, D], F32R)
        nc.sync.dma_start(bvrow_t[:], bvrow[:])
        borow_t = cpool.tile([1, D], F32R)
        nc.sync.dma_start(borow_t[:], borow[:])
        bf2row_t = cpool.tile([1, D], F32R)
        nc.sync.dma_start(bf2row_t[:], bf2row[:])
        vneg_t = cpool.tile([128, NT * H], F32)
        nc.sync.dma_start(vneg_t[:], vneg[:])
        vpos_t = cpool.tile([128, NT * H], F32)
        nc.sync.dma_start(vpos_t[:], vpos[:])
        vpln_t = cpool.tile([128, NT * H], F32)
        nc.sync.dma_start(vpln_t[:], vpln[:])
        qpos_c = [cpool.tile([128, CH], F32, tag=f"qposc{c}", name=f"qposc{c}") for c in range(NQP)]
        qneg_c = [cpool.tile([128, CH], F32, tag=f"qnegc{c}", name=f"qnegc{c}") for c in range(NQP)]
        for c in range(NQP):
            nc.sync.dma_start(qpos_c[c][:], qposb[c])
            nc.sync.dma_start(qneg_c[c][:], qnegb[c])
        qpos_t = [qpos_c[c % NQP] for c in range(KC)]
        qneg_t = [qneg_c[c % NQP] for c in range(KC)]
        # diagonal masks: resident when uniform (1 head), streamed per
        # head otherwise
        dmpool = top.enter_context(
            tc.tile_pool(name="dmp", bufs=1 if uniform else 2))

        def dm_fetch(h):
            tl = [dmpool.tile([128, CH], F32, tag=f"dmj{j}", name=f"dmj{j}")
                  for j in range(2)]
            for j in range(2):
                nc.sync.dma_start(tl[j][:], dmask[h % NHM, j])
            return tl

        dmask_uni = dm_fetch(0) if uniform else None
        eps_t = cpool.tile([128, 1], F32)
        nc.gpsimd.memset(eps_t[:], 1e-5)
        # warm the ACT LUT tables while input DMAs run
        warm_t = cpool.tile([1, 1], F32)
        for fn in (AF.Copy, AF.Sqrt, AF.Exp, AF.Relu, AF.Gelu):
            nc.scalar.activation(warm_t[:], eps_t[0:1, :], fn)
        ones1 = cpool.tile([1, 128], F32)
        nc.gpsimd.memset(ones1[:], 1.0)
        ones1r = cpool.tile([1, 128], F32R)
        nc.vector.tensor_copy(ones1r[:], ones1[:])

        # weights resident for attention phase
        wq_t = [cpool.tile([128, D], F32R, tag=f"wq{k}", name=f"wq{k}") for k in range(KC)]
        wk_t = [cpool.tile([128, D], F32R, tag=f"wk{k}", name=f"wk{k}") for k in range(KC)]
        wv_t = [cpool.tile([128, D], F32R, tag=f"wv{k}", name=f"wv{k}") for k in range(KC)]
        wo_t = [cpool.tile([128, D], F32R, tag=f"wo{k}", name=f"wo{k}") for k in range(KC)]
        for k in range(KC):
            nc.sync.dma_start(wq_t[k][:], wq[128 * k:128 * (k + 1), :])
            nc.sync.dma_start(wk_t[k][:], wk[128 * k:128 * (k + 1), :])
            nc.sync.dma_start(wv_t[k][:], wv[128 * k:128 * (k + 1), :])
            nc.sync.dma_start(wo_t[k][:], wo[128 * k:128 * (k + 1), :])

        # FFN1 weight stream pool at top scope so the first spans can be
        # prefetched long before phase E needs them.
        w1pool = top.enter_context(
            tc.tile_pool(name="w1s", bufs=2 if uniform else 1))
        w1tiles = {}
        def w1_fetch(qh, fs):
            key = (qh, fs)
            if key in w1tiles:
                return w1tiles[key]
            tl = [w1pool.tile([128, 512], F32R, tag=f"w1k{k}", name=f"w1k{k}")
                  for k in range(KC)]
            for k in range(KC):
                nc.sync.dma_start(
                    tl[k][:], w1[128 * k:128 * (k + 1),
                                 128 * fs:128 * fs + 512])
            w1tiles[key] = tl
            return tl
        w1_fetch(0, 0)   # prefetch the first span now

        x2_t = [x2pool.tile([128, D], F32, tag=f"x2_{q}", name=f"x2_{q}") for q in range(NQB)]
        xs2T = [x2pool.tile([128, TPC], F32R, tag=f"xs2T{c}", name=f"xs2T{c}") for c in range(KC)]

        with ExitStack() as attn:
            apool = attn.enter_context(tc.tile_pool(name="attn", bufs=1))
            spool = attn.enter_context(tc.tile_pool(
                name="st", bufs=2 * ND + (4 if uniform else 0)))
            tpool = attn.enter_context(tc.tile_pool(name="tmp", bufs=4))
            zpool = attn.enter_context(
                tc.tile_pool(name="ztmp", bufs=4 if uniform else 2))
            psS = attn.enter_context(
                tc.tile_pool(name="psS", bufs=3, space="PSUM"))
            psAV = attn.enter_context(
                tc.tile_pool(name="psAV", bufs=2, space="PSUM"))
            psPJ = attn.enter_context(
                tc.tile_pool(name="psPJ", bufs=3, space="PSUM"))

            # ---- phase A: LN1 + transpose ------------------------------
            def layer_norm(src, xs_out):
                st6 = tpool.tile([128, 6], F32, tag="ln_st6", name="ln_st6")
                nc.vector.bn_stats(st6[:], src[:])
                mv = tpool.tile([128, 2], F32, tag="ln_mv", name="ln_mv")
                nc.vector.bn_aggr(mv[:], st6[:])
                sd = tpool.tile([128, 1], F32, tag="ln_sd", name="ln_sd")
                nc.scalar.activation(sd[:], mv[:, 1:2], AF.Sqrt, bias=eps_t[:])
                rr = tpool.tile([128, 1], F32, tag="ln_rr", name="ln_rr")
                nc.vector.reciprocal_approx_fast(rr[:], sd[:])
                nc.vector.tensor_scalar(xs_out[:], src[:], mv[:, 0:1], rr[:],
                                        AluOpType.subtract, AluOpType.mult)

            xsT = [apool.tile([128, L], F32R, tag=f"xsT{c}", name=f"xsT{c}") for c in range(KC)]
            for i in range(NT):
                xs = tpool.tile([128, D], F32, tag="ln_xs", name="ln_xs",
                                bufs=2 if uniform else 1)
                layer_norm(x_t[i], xs)
                for c in range(KC):
                    pt = psS.tile([128, CH], F32, tag="s", name="s")
                    nc.tensor.transpose(pt[:, 0:128],
                                        xs[:, 128 * c:128 * (c + 1)], id_t[:])
                    if (i + c) % 2 == 0:
                        nc.scalar.copy(xsT[c][:, 128 * i:128 * (i + 1)],
                                       pt[:, 0:128])
                    else:
                        nc.vector.tensor_copy(xsT[c][:, 128 * i:128 * (i + 1)],
                                              pt[:, 0:128])

            # ---- phase B: K^T, Qp/Qf, V variants -----------------------
            def feat_from_psum(ps, bias_col, out_ap, span):
                """out = elu(ps + bias) + 1 elementwise; out f32r."""
                m = tpool.tile([128, 2 * CH], F32, tag="feat_m", name="feat_m", bufs=2 if uniform else 1)
                nc.vector.tensor_scalar(m[:, :span], ps[:, :span], bias_col,
                                        0.0, AluOpType.add, AluOpType.min)
                r = tpool.tile([128, 2 * CH], F32, tag="feat_r", name="feat_r", bufs=2 if uniform else 1)
                nc.scalar.activation(r[:, :span], ps[:, :span], AF.Relu,
                                     bias=bias_col)
                e = tpool.tile([128, 2 * CH], F32, tag="feat_e", name="feat_e", bufs=2 if uniform else 1)
                nc.scalar.activation(e[:, :span], m[:, :span], AF.Exp)
                nc.vector.tensor_add(out_ap, e[:, :span], r[:, :span])

            kT = [apool.tile([128, L], F32R, tag=f"kT{c}", name=f"kT{c}") for c in range(KC)]
            for c in range(KC):
                for s in range(0, L, 2 * CH):
                    span = min(2 * CH, L - s)
                    ps = psPJ.tile([128, D], F32, tag="pj", name="pj")
                    for k in range(KC):
                        nc.tensor.matmul(
                            ps[:, :span],
                            wk_t[k][:, 128 * c:128 * (c + 1)],
                            xsT[k][:, s:s + span],
                            start=(k == 0), stop=(k == KC - 1))
                    feat_from_psum(ps, bk_t[:, c:c + 1],
                                   kT[c][:, s:s + span], span)

            qp = [apool.tile([128, TPC], F32R, tag=f"qp{c}", name=f"qp{c}") for c in range(KC)]
            qf = [apool.tile([128, TPC], F32R, tag=f"qf{c}", name=f"qf{c}") for c in range(KC)]
            for c in range(KC):
                ps = psPJ.tile([128, D], F32, tag="pj", name="pj")
                for k in range(KC):
                    nc.tensor.matmul(
                        ps[:],
                        wq_t[k][:, 128 * c:128 * (c + 1)],
                        xsT[k][:, W:W + TPC],
                        start=(k == 0), stop=(k == KC - 1))
                ft = tpool.tile([128, TPC], F32, tag="feat_q", name="feat_q",
                                bufs=2 if uniform else 1)
                feat_from_psum(ps, bq_t[:, c:c + 1], ft[:], TPC)
                for qi in range(NQC):
                    qs2 = slice(qi * CH, (qi + 1) * CH)
                    nc.vector.tensor_mul(qp[c][:, qs2], ft[:, qs2],
                                         qpos_t[c][:])
                    nc.vector.tensor_mul(qf[c][:, qs2], ft[:, qs2],
                                         qneg_t[c][:])

            # role maps for key 128-blocks
            pastkb, diagkb, futkb = set(), set(), set()
            for qi in range(NQC):
                s = W + qi * CH
                for j in range(nW):
                    pastkb.add((s - W + 128 * j) // 128)
                    futkb.add((s + CH + 128 * j) // 128)
                diagkb.add(s // 128)
                diagkb.add(s // 128 + 1)

            vaug = {}
            for i in range(NT):
                roles = []
                if i in pastkb:
                    roles.append(("n", vneg_t))
                if i in diagkb:
                    roles.append(("p", vpln_t))
                if i in futkb:
                    roles.append(("f", vpos_t))
                if not roles:
                    continue
                ps = psPJ.tile([128, D], F32, tag="pj", name="pj")
                for k in range(KC):
                    nc.tensor.matmul(ps[:],
                                     xsT[k][:, 128 * i:128 * (i + 1)],
                                     wv_t[k][:],
                                     start=(k == 0),
                                     stop=(nobias and k == KC - 1))
                if nobias:
                    pass
                else:
                    nc.tensor.matmul(ps[:], ones1r[:], bvrow_t[:],
                                     start=False, stop=True)
                for rname, vsc in roles:
                    va = apool.tile([128, H * (HD + 1)], F32R,
                                    tag=f"vaug_{rname}{i}",
                                    name=f"vaug_{rname}{i}")
                    vaug[(rname, i)] = va
                    if uniform:
                        sc = vsc[:, i * H:i * H + 1]
                        # all heads in one strided op: out cols (65h..65h+63)
                        nc.scalar.activation(
                            va[:].rearrange("p (h d) -> p h d", h=H)[:, :, 0:64],
                            ps[:].rearrange("p (h d) -> p h d", h=H),
                            AF.Copy, scale=sc)
                        nc.scalar.copy(
                            va[:].rearrange("p (h d) -> p h d", h=H)[:, :, 64:65],
                            sc.broadcast_to((128, H, 1)))
                    else:
                        for h in range(H):
                            sc = vsc[:, i * H + h:i * H + h + 1]
                            nc.scalar.activation(va[:, h * 65:h * 65 + 64],
                                                 ps[:, h * 64:(h + 1) * 64],
                                                 AF.Copy, scale=sc)
                            nc.scalar.copy(va[:, h * 65 + 64:h * 65 + 65], sc)

            # ---- phase C: attention per head ---------------------------
            # Per (qi): past kbs use (Qp, Vneg), the two diagonal kbs use
            # (Qp, masked, Vplain), future kbs use (Qf, Vpos).  QK matmuls
            # with the same (kb, Q-variant) across adjacent q-chunks merge
            # into one wider matmul.
            plans = []                    # per qi: list of (kb, vid, role, mi)
            for qi in range(NQC):
                s = W + qi * CH
                plan = []
                for j in range(nW):
                    plan.append(((s - W) // 128 + j, 0, "n", None))
                plan.append((s // 128, 0, "p", 0))
                plan.append((s // 128 + 1, 0, "p", 1))
                for j in range(nW):
                    plan.append(((s + CH) // 128 + j, 1, "f", None))
                plans.append(plan)
            # merged QK jobs: (kb, vid) -> list of qi (ascending, contiguous)
            jobs = {}
            for qi in range(NQC):
                for kb, vid, role, mi in plans[qi]:
                    jobs.setdefault((kb, vid), []).append(qi)

            numT = [apool.tile([128, TPC], F32R, tag=f"numT{c}", name=f"numT{c}")
                    for c in range(KC)]
            ncopy = 0
            for h in range(H):
                c = h // 2
                po = (h % 2) * 64
                dmh = dmask_uni if uniform else dm_fetch(h)
                sts = {}                  # (qi, kb, role) -> st AP
                nsp = 0
                for (kb, vid), qis in jobs.items():
                    qv = (qp, qf)[vid]
                    qlo, qhi = qis[0] * CH, (qis[-1] + 1) * CH
                    wdt = qhi - qlo
                    if nsp % 2 == 0:
                        pss = psS.tile([128, 2 * CH], F32, tag="s", name="s")
                    else:
                        pss = psPJ.tile([128, D], F32, tag="pj", name="pj")
                    nsp += 1
                    nc.tensor.matmul(
                        pss[:, :wdt],
                        kT[c][po:po + 64, 128 * kb:128 * (kb + 1)],
                        qv[c][po:po + 64, qlo:qhi],
                        start=True, stop=True)
                    for qi in qis:
                        role_mi = [(r, m) for kb2, v2, r, m in plans[qi]
                                   if kb2 == kb and v2 == vid]
                        for role, mi in role_mi:
                            st = spool.tile([128, CH], F32R, tag="st",
                                            name="st")
                            src = pss[:, qi * CH - qlo:qi * CH - qlo + CH]
                            if mi is None:
                                if ncopy % 2 == 0:
                                    nc.scalar.copy(st[:], src)
                                else:
                                    nc.vector.tensor_copy(st[:], src)
                                ncopy += 1
                            else:
                                nc.vector.tensor_mul(st[:], src, dmh[mi][:])
                            sts[(qi, kb, role)] = st

                for qi in range(NQC):
                    qsl = slice(qi * CH, (qi + 1) * CH)
                    plan = plans[qi]
                    pav = psAV.tile([65, CH], F32, tag="av", name="av")
                    for idx, (kb, vid, role, mi) in enumerate(plan):
                        nc.tensor.matmul(
                            pav[:],
                            vaug[(role, kb)][:, h * 65:(h + 1) * 65],
                            sts[(qi, kb, role)][:],
                            start=(idx == 0), stop=(idx == len(plan) - 1))

                    zmax = zpool.tile([1, CH], F32, tag="zmax", name="zmax")
                    nc.vector.tensor_scalar_max(zmax[:], pav[64:65, :], 1e-6)
                    zr = zpool.tile([1, CH], F32, tag="zr", name="zr")
                    nc.vector.reciprocal_approx_fast(zr[:], zmax[:])
                    zb = zpool.tile([64, CH], F32, tag="zbs", name="zbs")
                    nc.gpsimd.partition_broadcast(zb[:], zr[:])
                    nc.vector.tensor_mul(numT[c][po:po + 64, qsl],
                                         pav[0:64, :], zb[:])

            # ---- phase D: O-proj + residual + LN2 ----------------------
            for q in range(NQB):
                ql = slice(128 * q, 128 * (q + 1))
                ps = psPJ.tile([128, D], F32, tag="pj", name="pj")
                for k in range(KC):
                    nc.tensor.matmul(ps[:], numT[k][:, ql], wo_t[k][:],
                                     start=(k == 0),
                                     stop=(nobias and k == KC - 1))
                if not nobias:
                    nc.tensor.matmul(ps[:], ones1r[:], borow_t[:],
                                     start=False, stop=True)
                xres = x_t[(W + 128 * q) // 128]
                nc.vector.tensor_add(x2_t[q][:], ps[:], xres[:])

                xs2 = tpool.tile([128, D], F32, tag="ln_xs", name="ln_xs",
                                 bufs=2 if uniform else 1)
                layer_norm(x2_t[q], xs2)
                for c in range(KC):
                    pt = psS.tile([128, CH], F32, tag="s", name="s")
                    nc.tensor.transpose(pt[:, 0:128],
                                        xs2[:, 128 * c:128 * (c + 1)], id_t[:])
                    if (q + c) % 2 == 0:
                        nc.scalar.copy(xs2T[c][:, ql], pt[:, 0:128])
                    else:
                        nc.vector.tensor_copy(xs2T[c][:, ql], pt[:, 0:128])

        # ---- phase E: FFN ----------------------------------------------
        with ExitStack() as ffn:
            fpool = ffn.enter_context(tc.tile_pool(name="ffn", bufs=1))
            psF = ffn.enter_context(
                tc.tile_pool(name="psF", bufs=3, space="PSUM"))
            psF2 = ffn.enter_context(
                tc.tile_pool(name="psF2", bufs=3, space="PSUM"))

            w2t = [fpool.tile([128, D], F32R, tag=f"w2f{f}", name=f"w2f{f}")
                   for f in range(FFC)]
            h1gT = [fpool.tile([128, TPC], F32R, tag=f"h1gT{f}", name=f"h1gT{f}")
                    for f in range(FFC)]
            # FFN1 in query halves of 256 so it can start as soon as the
            # first half of xs2T is written in phase D.
            for qh in range(2):
                if qh == 1:
                    # w2 prefetch after the half-0 w1 stream is in flight
                    for f in range(FFC):
                        nc.sync.dma_start(w2t[f][:],
                                          w2[128 * f:128 * (f + 1), :])
                hsl = slice(CH * qh, CH * (qh + 1))
                for fs in range(0, FFC, 4):      # ff spans of 512
                    w1t = w1_fetch(qh, fs)
                    for fo in range(4):
                        f = fs + fo
                        ps = psF.tile([128, TPC], F32, tag="f1", name="f1")
                        for k in range(KC):
                            nc.tensor.matmul(
                                ps[:, :CH], w1t[k][:, 128 * fo:128 * (fo + 1)],
                                xs2T[k][:, hsl],
                                start=(k == 0), stop=(k == KC - 1))
                        nc.scalar.activation(h1gT[f][:, hsl], ps[:, :CH],
                                             AF.Gelu, bias=bf1_t[:, f:f + 1])

            for q in range(NQB):
                ql = slice(128 * q, 128 * (q + 1))
                ps2 = psF2.tile([128, D], F32, tag="f2", name="f2")
                for f in range(FFC):
                    nc.tensor.matmul(ps2[:],
                                     h1gT[f][:, ql],
                                     w2t[f][:],
                                     start=(f == 0),
                                     stop=(nobias and f == FFC - 1))
                if not nobias:
                    nc.tensor.matmul(ps2[:], ones1r[:], bf2row_t[:],
                                     start=False, stop=True)
                o = fpool.tile([128, D], F32, tag="fout", name="fout", bufs=2)
                nc.vector.tensor_add(o[:], ps2[:], x2_t[q][:])
                nc.sync.dma_start(out[ql, :], o[:])

    nc.compile()
    return nc


def _get_nc(W, uniform, nobias):
    key = (W, uniform, nobias)
    if key not in _BUILD_CACHE:
        _BUILD_CACHE[key] = _build(W, uniform, nobias)
    return _BUILD_CACHE[key]


# ---------------------------------------------------------------------------
# host side
# ---------------------------------------------------------------------------

def kernel(x, mask, wq, wk, wv, wo, bo, g1, b1, g2, b2, w1, bf1, w2, bf2,
           decay_logit):
    x = np.asarray(x, np.float32)
    mask = np.asarray(mask)
    g1 = np.asarray(g1, np.float64)
    b1 = np.asarray(b1, np.float64)
    g2 = np.asarray(g2, np.float64)
    b2 = np.asarray(b2, np.float64)

    d64 = 1.0 / (1.0 + np.exp(-np.asarray(decay_logit, np.float64)))
    d64 = np.clip(d64, 1e-8, None)
    dmax = float(d64.max())
    # Band width: the attention kernel truncates at |i-j| > ~W.  The omitted
    # mass relative to the normalizer z is ~ d^(W+1)/(1+d); keep it under
    # 1e-5, well below the fp32r matmul noise (~1e-4).
    if dmax >= 1.0 - 1e-12:
        W = 512  # decay ~1: widest supported band
    else:
        need = math.log(1e-5 * (1.0 + dmax)) / math.log(dmax)
        W = max(128, 128 * math.ceil(need / 128))
        W = min(W, 512)
    nW = W // 128
    L = TPC + 2 * W
    uniform = bool(np.all(d64 == d64[0]))
    nobias = bool(np.all(b1 == 0) and np.all(np.asarray(bo) == 0)
                  and np.all(np.asarray(bf2) == 0))

    nc = _get_nc(W, uniform, nobias)

    # folded weights (fp32r pre-rounded)
    wq_f = _np_fp32r(g1[:, None] * np.asarray(wq, np.float64))
    wk_f = _np_fp32r(g1[:, None] * np.asarray(wk, np.float64))
    wv_f = _np_fp32r(g1[:, None] * np.asarray(wv, np.float64))
    wo_f = _np_fp32r(wo)
    w1_f = _np_fp32r(g2[:, None] * np.asarray(w1, np.float64))
    w2_f = _np_fp32r(w2)
    bq_h = (b1 @ np.asarray(wq, np.float64)).astype(np.float32)
    bk_h = (b1 @ np.asarray(wk, np.float64)).astype(np.float32)
    bv_h = _np_fp32r((b1 @ np.asarray(wv, np.float64)).astype(np.float32))
    bo_h = _np_fp32r(np.asarray(bo, np.float32))
    bf1_h = (b2 @ np.asarray(w1, np.float64) +
             np.asarray(bf1, np.float64)).astype(np.float32)
    bf2_h = _np_fp32r(np.asarray(bf2, np.float32))

    # decay-derived tables
    ld = np.log(d64)                                    # (H,)
    di = np.arange(CH, dtype=np.float64)
    NQP = 1 if uniform else KC
    NHM = 1 if uniform else H
    qposb = np.zeros((NQP, 128, CH), np.float32)
    qnegb = np.zeros((NQP, 128, CH), np.float32)
    for c in range(NQP):
        for hh in range(2):
            h = 2 * c + hh
            qposb[c, 64 * hh:64 * hh + 64, :] = np.exp(ld[h] * di)[None, :]
            qnegb[c, 64 * hh:64 * hh + 64, :] = np.exp(-ld[h] * di)[None, :]
    if uniform:
        qposb[:, 64:, :] = qposb[:, :64, :]
        qnegb[:, 64:, :] = qnegb[:, :64, :]
    # diagonal-zone mask: m[dk, di] = d^{|di-dk| - di}, dk in [0,256)
    dk = np.arange(CH, dtype=np.float64)
    dmask = np.zeros((NHM, 2, 128, CH), np.float32)
    for h in range(NHM):
        m = np.exp(ld[h] * (np.abs(di[None, :] - dk[:, None]) - di[None, :]))
        dmask[h, 0] = m[:128, :]
        dmask[h, 1] = m[128:, :]

    in_maps = []
    for core in range(NCORES):
        b = core // SH
        t0 = (core % SH) * TPC
        lo, hi = t0 - W, t0 + TPC + W
        xs = np.zeros((L, D), np.float32)
        pad = np.zeros((L,), np.float64)
        glo, ghi = max(lo, 0), min(hi, T)
        xs[glo - lo:ghi - lo] = x[b, glo:ghi]
        pad[glo - lo:ghi - lo] = (~mask[b, glo:ghi]).astype(np.float64)

        vneg = np.zeros((H, L), np.float32)
        vpos = np.zeros((H, L), np.float32)
        vpln = np.zeros((H, L), np.float32)
        for h in range(H):
            vn = np.zeros(L)
            vp = np.zeros(L)
            for qi in range(NQC):
                s = W + qi * CH
                jj = np.arange(s - W, s)
                vn[jj] = np.exp(ld[h] * (s - jj))
                jj = np.arange(s + CH, s + CH + W)
                vp[jj] = np.exp(ld[h] * (CH + jj - (s + CH)))
            vneg[h] = (vn * pad).astype(np.float32)
            vpos[h] = (vp * pad).astype(np.float32)
            vpln[h] = pad.astype(np.float32)
        def _vlayout(a):
            return np.ascontiguousarray(
                a.reshape(H, L // 128, 128).transpose(2, 1, 0)
                 .reshape(128, (L // 128) * H))
        vneg, vpos, vpln = _vlayout(vneg), _vlayout(vpos), _vlayout(vpln)

        in_maps.append({
            "xsl": xs,
            "wq": wq_f, "wk": wk_f, "wv": wv_f, "wo": wo_f,
            "w1": w1_f, "w2": w2_f,
            "bq": bq_h, "bk": bk_h,
            "bvrow": bv_h.reshape(1, D), "borow": bo_h.reshape(1, D),
            "bf1": bf1_h, "bf2row": bf2_h.reshape(1, D),
            "vneg": vneg, "vpos": vpos, "vpln": vpln,
            "qposb": qposb, "qnegb": qnegb, "dmask": dmask,
            "ident": np.eye(128, dtype=np.float32),
        })

    res = bass_utils.run_bass_kernel_spmd(nc, in_maps,
                                          core_ids=list(range(NCORES)))
    out = np.empty((B, T, D), np.float32)
    for core in range(NCORES):
        b = core // SH
        t0 = (core % SH) * TPC
        out[b, t0:t0 + TPC] = res.results[core]["out"]

    # Degenerate-mask patch: a query whose entire +-W neighbourhood is
    # masked has z ~ clip floor (1e-6); the reference output is then
    # dominated by out-of-band keys that the banded kernel truncates.
    # Recompute those rows exactly on the host (never triggers for an
    # all-False mask).
    if mask.any():
        idx = np.arange(T)
        for b_ in range(B):
            keep = ~np.asarray(mask[b_])
            if keep.all():
                continue
            kpos = idx[keep]
            if len(kpos) == 0:
                dist = np.full(T, T)
            else:
                ins = np.searchsorted(kpos, idx)
                left = np.where(ins > 0, idx - kpos[np.clip(ins - 1, 0,
                                                            len(kpos) - 1)], T)
                right = np.where(ins < len(kpos),
                                 kpos[np.clip(ins, 0, len(kpos) - 1)] - idx, T)
                dist = np.minimum(left, right)
            need = dist > (W - 32)
            if need.any():
                out[b_, need] = _exact_rows(
                    np.asarray(x[b_], np.float64), keep.astype(np.float64),
                    np.where(need)[0], wq, wk, wv, wo, bo, g1, b1, g2, b2,
                    w1, bf1, w2, bf2, d64)
    return out


def _exact_rows(xb, pad, rows, wq, wk, wv, wo, bo, g1, b1, g2, b2,
                w1, bf1, w2, bf2, d64):
    """Reference math (float64) for the given query rows of one batch."""
    import scipy.special as _sp

    def ln(z, g, b):
        mu = z.mean(-1, keepdims=True)
        var = ((z - mu) ** 2).mean(-1, keepdims=True)
        return (z - mu) / np.sqrt(var + 1e-5) * g + b

    wq = np.asarray(wq, np.float64); wk = np.asarray(wk, np.float64)
    wv = np.asarray(wv, np.float64); wo = np.asarray(wo, np.float64)
    w1 = np.asarray(w1, np.float64); w2 = np.asarray(w2, np.float64)
    xn = ln(xb, np.asarray(g1, np.float64), np.asarray(b1, np.float64))

    def feat(z):
        return np.where(z > 0, z + 1.0, np.exp(np.minimum(z, 0.0)))

    Q = feat((xn[rows] @ wq).reshape(len(rows), H, HD))       # (R,H,hd)
    K = feat((xn @ wk).reshape(T, H, HD)) * pad[:, None, None]
    V = (xn @ wv).reshape(T, H, HD) * pad[:, None, None]
    dist = np.abs(rows[:, None] - np.arange(T)[None, :]).astype(np.float64)
    M = np.exp(dist[None] * np.log(d64)[:, None, None])       # (H,R,T)
    A = np.einsum('rhd,jhd->hrj', Q, K) * M
    z = np.clip(A.sum(-1, keepdims=True), 1e-6, None)
    o = np.einsum('hrj,jhd->rhd', A / z, V).reshape(len(rows), D)
    o = o @ wo + np.asarray(bo, np.float64)
    x2 = xb[rows] + o
    xn2 = ln(x2, np.asarray(g2, np.float64), np.asarray(b2, np.float64))
    h1 = xn2 @ w1 + np.asarray(bf1, np.float64)
    gl = 0.5 * h1 * (1.0 + _sp.erf(h1 / np.sqrt(2.0)))
    ffn = gl @ w2 + np.asarray(bf2, np.float64)
    return (x2 + ffn).astype(np.float32)



# revision 18
# speedup vs baseline: 1.4738x; 1.4738x over previous
"""Trainium2 Bass kernel for a bidirectional linear-attention transformer layer.

Contract: kernel(**inputs) takes the FULL unsharded inputs (as produced by the
problem's setup_inputs()) and returns the FULL (B, T, D) float32 output.

Sharding: token-parallel across 8 NeuronCores. Core c handles batch c//4,
query tokens [ (c%4)*512, (c%4)*512+512 ), with a +-W token halo (W=128 for
decay=0.9). The decay mask d^|i-j| factorizes per 256-token query chunk into
  - a Q-side scale d^{+di} (past/diag) or d^{-di} (future), baked into two
    pre-scaled copies of Q,
  - a K-side scale folded into V rows and into an extra "ones" column of V
    that simultaneously produces the normalization denominator z,
  - an elementwise 256x256 mask for the diagonal zone only.
All heavy matmuls run in bf16 (fp32 PSUM accumulate).  The residual path and
all statistics (LN, normalizers) stay fp32.

Structure (per core):
  A  LN1 + PE transposes            -> xsT (bf16, [d, tok])
  B  K^T features, Qp/Qf, V+decay   -> kT, qp, qf, vaug (bf16)
  C1 QK scores for ALL heads        -> st tiles (bf16, SBUF resident)
  C2 AV chains for ALL heads        -> numT (normalized, bf16)
  D  O-proj + residual + LN2        -> x2 (f32), xs2T (bf16)
  E  FFN1 + gelu + FFN2             -> out
C1/C2 are bulk stages (not per-head zigzags) so the PE stream stays dense and
cross-engine latency is hidden by other heads' work.
"""

import math
import os
from contextlib import ExitStack

import numpy as np
import ml_dtypes

import concourse.bacc as bacc
import concourse.mybir as mybir
import concourse.tile as tile
from concourse import bass_utils
from concourse.alu_op_type import AluOpType

F32 = mybir.dt.float32
BF16 = mybir.dt.bfloat16
AF = mybir.ActivationFunctionType

B, T, D, H, FF = 2, 2048, 512, 8, 2048
HD = D // H          # 64
NCORES = 8
SH = 4               # token shards per batch
TPC = T // SH        # 512 query tokens per core
CH = 256             # query chunk width
NQC = TPC // CH      # 2 query chunks per core
KC = D // 128        # 4 contraction chunks over D
FFC = FF // 128      # 16 ff chunks

# ---------------------------------------------------------------------------
# device kernel build (depends only on W, uniform, nobias)
# ---------------------------------------------------------------------------

_BUILD_CACHE = {}


def _cs_offsets():
    # fp32 const pack #1 (small, early): bq, bk, bf1, vneg, vpos, vpln
    o = {}
    o["bq"] = 0
    o["bk"] = 4
    o["bf1"] = 8
    o["vn"] = 24
    return o


def _build(W, uniform, nobias):
    nW = W // 128            # past/future 128-blocks per q-chunk
    L = TPC + 2 * W          # local tokens per core
    NT = L // 128            # 128-token tiles per core
    NQB = TPC // 128         # 4 query 128-blocks
    NHM = 1 if uniform else H    # distinct decay tables needed
    NQP = 1 if uniform else KC

    CS_COLS = 24 + 3 * NT * H                 # fp32 pack 1
    O_VN = 24
    O_QP = 0                                  # fp32 pack 2 offsets
    O_QN = NQP * CH
    O_DM = 2 * NQP * CH
    CB_COLS = O_DM + NHM * 2 * CH

    nc = bacc.Bacc("TRN2", target_bir_lowering=False, debug=False)

    def dram_in(name, shape, dt=F32):
        return nc.dram_tensor(name, shape, dt, kind="ExternalInput").ap()

    # DMA issue order == declaration/use order below (HWDGE FIFO):
    xbig_a = dram_in("xh0", [128, (NT // 2) * D])     # x tokens, first half
    xbig_b = dram_in("xh1", [128, (NT - NT // 2) * D])
    cs_d = dram_in("cs", [128, CS_COLS])              # small fp32 consts
    wa_d = dram_in("wa", [128, 2 * KC * 512 + 128], BF16)   # wk|wq|ident
    cb_d = dram_in("cb", [128, CB_COLS])              # qpos|qneg|dmask
    wb_d = dram_in("wb", [128, 2 * KC * 512], BF16)   # wv|wo
    w1_d = dram_in("w1", [128, KC * FF], BF16)
    w2_d = dram_in("w2", [128, FFC * D], BF16)
    if not nobias:
        brow_d = dram_in("brow", [1, 3 * D], BF16)    # bv|bo|bf2 rows

    out = nc.dram_tensor("out", [TPC, D], F32, kind="ExternalOutput").ap()

    dbg = os.environ.get("KDBG") == "1"
    dbg_d = {}
    if dbg:
        for nm, shape in [("d_xsT", [128, L]), ("d_kT", [128, L]),
                          ("d_qp", [128, TPC]), ("d_qf", [128, TPC]),
                          ("d_vaug", [128, H * (HD + 1)]),
                          ("d_st", [128, CH]), ("d_numT", [128, TPC]),
                          ("d_x2", [128, D]), ("d_h1", [128, TPC])]:
            dbg_d[nm] = nc.dram_tensor(nm, shape, F32,
                                       kind="ExternalOutput").ap()

    with tile.TileContext(nc) as tc, ExitStack() as top:
        cpool = top.enter_context(tc.tile_pool(name="const", bufs=1))
        xpool = top.enter_context(tc.tile_pool(name="x", bufs=1))
        x2pool = top.enter_context(tc.tile_pool(name="x2", bufs=1))

        # ---- input DMAs (one per packed tensor) ------------------------
        xh0 = xpool.tile([128, (NT // 2) * D], F32, tag="xh0", name="xh0")
        nc.sync.dma_start(xh0[:], xbig_a[:])
        xh1 = xpool.tile([128, (NT - NT // 2) * D], F32, tag="xh1", name="xh1")
        nc.sync.dma_start(xh1[:], xbig_b[:])

        def x_t(i):
            if i < NT // 2:
                return xh0[:, i * D:(i + 1) * D]
            j = i - NT // 2
            return xh1[:, j * D:(j + 1) * D]

        cs_t = cpool.tile([128, CS_COLS], F32, tag="cs", name="cs")
        nc.sync.dma_start(cs_t[:], cs_d[:])
        wa_t = cpool.tile([128, 2 * KC * 512 + 128], BF16, tag="wa", name="wa")
        nc.sync.dma_start(wa_t[:], wa_d[:])
        cb_t = cpool.tile([128, CB_COLS], F32, tag="cb", name="cb")
        nc.sync.dma_start(cb_t[:], cb_d[:])
        wb_t = cpool.tile([128, 2 * KC * 512], BF16, tag="wb", name="wb")
        nc.sync.dma_start(wb_t[:], wb_d[:])
        w1_t = cpool.tile([128, KC * FF], BF16, tag="w1", name="w1")
        nc.sync.dma_start(w1_t[:], w1_d[:])
        w2_t = cpool.tile([128, FFC * D], BF16, tag="w2", name="w2")
        nc.sync.dma_start(w2_t[:], w2_d[:])
        if not nobias:
            brow_t = cpool.tile([1, 3 * D], BF16, tag="brow", name="brow")
            nc.sync.dma_start(brow_t[:], brow_d[:])
            ones1 = cpool.tile([1, 128], F32, tag="on1", name="on1")
            nc.gpsimd.memset(ones1[:], 1.0)
            ones1b = cpool.tile([1, 128], BF16, tag="on1b", name="on1b")
            nc.vector.tensor_copy(ones1b[:], ones1[:])

        def wk_s(k, c):
            return wa_t[:, k * 512 + 128 * c:k * 512 + 128 * (c + 1)]

        def wq_s(k, c):
            return wa_t[:, 2048 + k * 512 + 128 * c:2048 + k * 512 + 128 * (c + 1)]

        id_t = wa_t[:, 4096:4224]
        def wv_s(k):
            return wb_t[:, k * 512:(k + 1) * 512]

        def wo_s(k):
            return wb_t[:, 2048 + k * 512:2048 + k * 512 + 512]

        bq_c = lambda c: cs_t[:, c:c + 1]
        bk_c = lambda c: cs_t[:, 4 + c:5 + c]
        bf1_c = lambda f: cs_t[:, 8 + f:9 + f]
        vsc = {"n": lambda i, h: cs_t[:, O_VN + i * H + h:O_VN + i * H + h + 1],
               "p": lambda i, h: cs_t[:, O_VN + 2 * NT * H + i * H + h:
                                      O_VN + 2 * NT * H + i * H + h + 1],
               "f": lambda i, h: cs_t[:, O_VN + NT * H + i * H + h:
                                      O_VN + NT * H + i * H + h + 1]}
        qpos_s = lambda c: cb_t[:, O_QP + (c % NQP) * CH:O_QP + (c % NQP) * CH + CH]
        qneg_s = lambda c: cb_t[:, O_QN + (c % NQP) * CH:O_QN + (c % NQP) * CH + CH]
        dm_s = lambda h, j: cb_t[:, O_DM + ((h % NHM) * 2 + j) * CH:
                                 O_DM + ((h % NHM) * 2 + j) * CH + CH]

        eps_t = cpool.tile([128, 1], F32, tag="eps", name="eps")
        nc.gpsimd.memset(eps_t[:], 1e-5)
        # warm the ACT LUT tables while input DMAs run
        warm_t = cpool.tile([1, 1], F32, tag="warm", name="warm")
        for fn in (AF.Copy, AF.Sqrt, AF.Exp, AF.Relu, AF.Gelu):
            nc.scalar.activation(warm_t[:], eps_t[0:1, :], fn)
        # PE warm-up: keep the PE busy (and the HAM un-throttled) during the
        # initial DMA wait so real matmuls run at 2.4 GHz from the start.
        wrm = cpool.tile([128, 512], BF16, tag="wrm", name="wrm")
        nc.gpsimd.memset(wrm[:], 0.0)

        def dump(nm, ap):
            if not dbg:
                return
            t = x2pool.tile(list(ap.shape), F32, tag="dbg_" + nm,
                            name="dbg_" + nm)
            nc.vector.tensor_copy(t[:], ap)
            nc.sync.dma_start(dbg_d[nm][:], t[:])

        x2_t = [x2pool.tile([128, D], F32, tag=f"x2_{q}", name=f"x2_{q}")
                for q in range(NQB)]
        xs2T = [x2pool.tile([128, TPC], BF16, tag=f"xs2T{c}", name=f"xs2T{c}")
                for c in range(KC)]

        with ExitStack() as attn:
            apool = attn.enter_context(tc.tile_pool(name="attn", bufs=1))
            spool = attn.enter_context(tc.tile_pool(name="st", bufs=1))
            tpool = attn.enter_context(tc.tile_pool(name="tmp", bufs=4))
            zpool = attn.enter_context(tc.tile_pool(name="ztmp", bufs=4))
            psS = attn.enter_context(
                tc.tile_pool(name="psS", bufs=2, space="PSUM"))
            psPJ = attn.enter_context(
                tc.tile_pool(name="psPJ", bufs=2, space="PSUM"))
            psAV = attn.enter_context(
                tc.tile_pool(name="psAV", bufs=2, space="PSUM"))
            psT = attn.enter_context(
                tc.tile_pool(name="psT", bufs=2, space="PSUM"))

            npj = [0]

            def pj_tile():
                pool = (psS, psPJ)[npj[0] % 2]
                npj[0] += 1
                return pool.tile([128, 512], F32,
                                 tag="s" if pool is psS else "pj",
                                 name="s" if pool is psS else "pj")

            for _ in range(12):
                wps = pj_tile()
                nc.tensor.matmul(wps[:, 0:512], wrm[:, 0:128], wrm[:, 0:512],
                                 start=True, stop=True)

            # ---- phase A: LN1 + transpose ------------------------------
            def layer_norm(src, xs_out):
                st6 = tpool.tile([128, 6], F32, tag="ln_st6", name="ln_st6")
                nc.vector.bn_stats(st6[:], src)
                mv = tpool.tile([128, 2], F32, tag="ln_mv", name="ln_mv")
                nc.vector.bn_aggr(mv[:], st6[:])
                sd = tpool.tile([128, 1], F32, tag="ln_sd", name="ln_sd")
                nc.scalar.activation(sd[:], mv[:, 1:2], AF.Sqrt, bias=eps_t[:])
                rr = tpool.tile([128, 1], F32, tag="ln_rr", name="ln_rr")
                nc.vector.reciprocal_approx_fast(rr[:], sd[:])
                nc.vector.tensor_scalar(xs_out, src, mv[:, 0:1], rr[:],
                                        AluOpType.subtract, AluOpType.mult)

            xsT = [apool.tile([128, L], BF16, tag=f"xsT{c}", name=f"xsT{c}")
                   for c in range(KC)]
            for i in range(NT):
                xs = tpool.tile([128, D], BF16, tag="ln_xs", name="ln_xs",
                                bufs=2)
                layer_norm(x_t(i), xs[:])
                pt = psT.tile([128, 512], BF16, tag="tp", name="tp")
                for c in range(KC):
                    nc.tensor.transpose(pt[:, 128 * c:128 * (c + 1)],
                                        xs[:, 128 * c:128 * (c + 1)], id_t)
                    if (i + c) % 2 == 0:
                        nc.scalar.copy(xsT[c][:, 128 * i:128 * (i + 1)],
                                       pt[:, 128 * c:128 * (c + 1)])
                    else:
                        nc.vector.tensor_copy(xsT[c][:, 128 * i:128 * (i + 1)],
                                              pt[:, 128 * c:128 * (c + 1)])

            dump("d_xsT", xsT[0][:])

            # ---- phase B: K^T, Qp/Qf, V variants -----------------------
            def feat_from_psum(ps, bias_col, out_ap, span):
                """out = elu(ps + bias) + 1 elementwise; out bf16."""
                m = tpool.tile([128, 2 * CH], F32, tag="feat_m", name="feat_m",
                               bufs=2)
                nc.vector.tensor_scalar(m[:, :span], ps[:, :span], bias_col,
                                        0.0, AluOpType.add, AluOpType.min)
                r = tpool.tile([128, 2 * CH], F32, tag="feat_r", name="feat_r",
                               bufs=2)
                nc.scalar.activation(r[:, :span], ps[:, :span], AF.Relu,
                                     bias=bias_col)
                e = tpool.tile([128, 2 * CH], F32, tag="feat_e", name="feat_e",
                               bufs=2)
                nc.scalar.activation(e[:, :span], m[:, :span], AF.Exp)
                nc.vector.tensor_add(out_ap, e[:, :span], r[:, :span])

            kT = [apool.tile([128, L], BF16, tag=f"kT{c}", name=f"kT{c}")
                  for c in range(KC)]
            for c in range(KC):
                for s in range(0, L, 2 * CH):
                    span = min(2 * CH, L - s)
                    ps = pj_tile()
                    for k in range(KC):
                        nc.tensor.matmul(
                            ps[:, :span], wk_s(k, c), xsT[k][:, s:s + span],
                            start=(k == 0), stop=(k == KC - 1))
                    feat_from_psum(ps, bk_c(c), kT[c][:, s:s + span], span)

            dump("d_kT", kT[0][:])

            qp = [apool.tile([128, TPC], BF16, tag=f"qp{c}", name=f"qp{c}")
                  for c in range(KC)]
            qf = [apool.tile([128, TPC], BF16, tag=f"qf{c}", name=f"qf{c}")
                  for c in range(KC)]
            for c in range(KC):
                ps = pj_tile()
                for k in range(KC):
                    nc.tensor.matmul(
                        ps[:], wq_s(k, c), xsT[k][:, W:W + TPC],
                        start=(k == 0), stop=(k == KC - 1))
                ft = tpool.tile([128, TPC], F32, tag="feat_q", name="feat_q",
                                bufs=2)
                feat_from_psum(ps, bq_c(c), ft[:], TPC)
                for qi in range(NQC):
                    qs2 = slice(qi * CH, (qi + 1) * CH)
                    nc.vector.tensor_mul(qp[c][:, qs2], ft[:, qs2], qpos_s(c))
                    nc.vector.tensor_mul(qf[c][:, qs2], ft[:, qs2], qneg_s(c))

            dump("d_qp", qp[0][:])
            dump("d_qf", qf[0][:])

            # role maps for key 128-blocks
            pastkb, diagkb, futkb = set(), set(), set()
            for qi in range(NQC):
                s = W + qi * CH
                for j in range(nW):
                    pastkb.add((s - W + 128 * j) // 128)
                    futkb.add((s + CH + 128 * j) // 128)
                diagkb.add(s // 128)
                diagkb.add(s // 128 + 1)

            vaug = {}
            for i in range(NT):
                roles = []
                if i in pastkb:
                    roles.append("n")
                if i in diagkb:
                    roles.append("p")
                if i in futkb:
                    roles.append("f")
                if not roles:
                    continue
                ps = pj_tile()
                for k in range(KC):
                    nc.tensor.matmul(ps[:],
                                     xsT[k][:, 128 * i:128 * (i + 1)],
                                     wv_s(k),
                                     start=(k == 0),
                                     stop=(nobias and k == KC - 1))
                if not nobias:
                    nc.tensor.matmul(ps[:], ones1b[:], brow_t[:, 0:D],
                                     start=False, stop=True)
                for rname in roles:
                    va = apool.tile([128, H * (HD + 1)], BF16,
                                    tag=f"vaug_{rname}{i}",
                                    name=f"vaug_{rname}{i}")
                    vaug[(rname, i)] = va
                    if uniform:
                        sc = vsc[rname](i, 0)
                        nc.scalar.activation(
                            va[:].rearrange("p (h d) -> p h d", h=H)[:, :, 0:64],
                            ps[:].rearrange("p (h d) -> p h d", h=H),
                            AF.Copy, scale=sc)
                        nc.scalar.copy(
                            va[:].rearrange("p (h d) -> p h d", h=H)[:, :, 64:65],
                            sc.broadcast_to((128, H, 1)))
                    else:
                        for h in range(H):
                            sc = vsc[rname](i, h)
                            nc.scalar.activation(va[:, h * 65:h * 65 + 64],
                                                 ps[:, h * 64:(h + 1) * 64],
                                                 AF.Copy, scale=sc)
                            nc.scalar.copy(va[:, h * 65 + 64:h * 65 + 65], sc)

            if dbg:
                dump("d_vaug", vaug[("p", W // 128)][:])

            # ---- phase C1: QK scores for ALL heads ---------------------
            plans = []                    # per qi: list of (kb, vid, role, mi)
            for qi in range(NQC):
                s = W + qi * CH
                plan = []
                for j in range(nW):
                    plan.append(((s - W) // 128 + j, 0, "n", None))
                plan.append((s // 128, 0, "p", 0))
                plan.append((s // 128 + 1, 0, "p", 1))
                for j in range(nW):
                    plan.append(((s + CH) // 128 + j, 1, "f", None))
                plans.append(plan)
            jobs = {}
            for qi in range(NQC):
                for kb, vid, role, mi in plans[qi]:
                    jobs.setdefault((kb, vid), []).append(qi)

            sts = {}
            ncopy = 0
            for h in range(H):
                c = h // 2
                po = (h % 2) * 64
                for (kb, vid), qis in jobs.items():
                    qv = (qp, qf)[vid]
                    qlo, qhi = qis[0] * CH, (qis[-1] + 1) * CH
                    wdt = qhi - qlo
                    pss = pj_tile()
                    nc.tensor.matmul(
                        pss[:, :wdt],
                        kT[c][po:po + 64, 128 * kb:128 * (kb + 1)],
                        qv[c][po:po + 64, qlo:qhi],
                        start=True, stop=True)
                    for qi in qis:
                        for kb2, v2, role, mi in plans[qi]:
                            if kb2 != kb or v2 != vid:
                                continue
                            st = spool.tile([128, CH], BF16,
                                            tag=f"st{h}_{qi}_{kb}_{role}",
                                            name=f"st{h}_{qi}_{kb}_{role}")
                            src = pss[:, qi * CH - qlo:qi * CH - qlo + CH]
                            if mi is None:
                                if ncopy % 2 == 0:
                                    nc.scalar.copy(st[:], src)
                                else:
                                    nc.vector.tensor_copy(st[:], src)
                                ncopy += 1
                            else:
                                nc.vector.tensor_mul(st[:], src, dm_s(h, mi))
                            sts[(h, qi, kb, role)] = st
                            if dbg and h == 0 and qi == 0 and role == "p" \
                                    and mi == 0:
                                dump("d_st", st[:])

            # ---- phase C2: AV chains for ALL heads ---------------------
            # z (the normalizer) is the 65th row of each AV chain.  The
            # banded z can only underflow if every in-band key of a query is
            # masked; those rows are recomputed exactly on the host (see
            # kernel()), so no clipping is needed here.
            numT = [apool.tile([128, TPC], BF16, tag=f"numT{c}",
                               name=f"numT{c}") for c in range(KC)]
            nav = 0
            for h in range(H):
                c = h // 2
                po = (h % 2) * 64
                for qi in range(NQC):
                    qsl = slice(qi * CH, (qi + 1) * CH)
                    plan = plans[qi]
                    sel = nav % 4
                    nav += 1
                    if sel < 2:
                        pav = psAV.tile([128, CH], F32, tag="av", name="av")
                    else:
                        pav = pj_tile()
                    for idx, (kb, vid, role, mi) in enumerate(plan):
                        nc.tensor.matmul(
                            pav[0:65, 0:CH],
                            vaug[(role, kb)][:, h * 65:(h + 1) * 65],
                            sts[(h, qi, kb, role)][:],
                            start=(idx == 0), stop=(idx == len(plan) - 1))
                    zmax = zpool.tile([1, CH], F32, tag="zmax", name="zmax")
                    nc.vector.tensor_scalar_max(zmax[:], pav[64:65, 0:CH],
                                                1e-6)
                    zr = zpool.tile([1, CH], F32, tag="zr", name="zr")
                    nc.vector.reciprocal_approx_fast(zr[:], zmax[:])
                    zb = zpool.tile([64, CH], F32, tag="zbs", name="zbs")
                    nc.gpsimd.partition_broadcast(zb[:], zr[:])
                    nc.vector.tensor_mul(numT[c][po:po + 64, qsl],
                                         pav[0:64, 0:CH], zb[:])

            dump("d_numT", numT[0][:])

            # ---- phase D: O-proj + residual + LN2 ----------------------
            for q in range(NQB):
                ql = slice(128 * q, 128 * (q + 1))
                ps = pj_tile()
                for k in range(KC):
                    nc.tensor.matmul(ps[:], numT[k][:, ql], wo_s(k),
                                     start=(k == 0),
                                     stop=(nobias and k == KC - 1))
                if not nobias:
                    nc.tensor.matmul(ps[:], ones1b[:], brow_t[:, D:2 * D],
                                     start=False, stop=True)
                xres = x_t((W + 128 * q) // 128)
                nc.vector.tensor_add(x2_t[q][:], ps[:], xres)

                xs2 = tpool.tile([128, D], BF16, tag="ln_xs2", name="ln_xs2",
                                 bufs=2)
                layer_norm(x2_t[q], xs2[:])
                pt = psT.tile([128, 512], BF16, tag="tp", name="tp")
                for c in range(KC):
                    nc.tensor.transpose(pt[:, 128 * c:128 * (c + 1)],
                                        xs2[:, 128 * c:128 * (c + 1)], id_t)
                    if (q + c) % 2 == 0:
                        nc.scalar.copy(xs2T[c][:, ql],
                                       pt[:, 128 * c:128 * (c + 1)])
                    else:
                        nc.vector.tensor_copy(xs2T[c][:, ql],
                                              pt[:, 128 * c:128 * (c + 1)])

        dump("d_x2", x2_t[0][:])

        # ---- phase E: FFN ----------------------------------------------
        with ExitStack() as ffn:
            fpool = ffn.enter_context(tc.tile_pool(name="ffn", bufs=1))
            psF = ffn.enter_context(
                tc.tile_pool(name="psF", bufs=4, space="PSUM"))
            psF2 = ffn.enter_context(
                tc.tile_pool(name="psF2", bufs=3, space="PSUM"))

            h1gT = [fpool.tile([128, TPC], BF16, tag=f"h1gT{f}",
                               name=f"h1gT{f}") for f in range(FFC)]

            def ffn2(q):
                ql = slice(128 * q, 128 * (q + 1))
                ps2 = psF2.tile([128, D], F32, tag="f2", name="f2")
                for f in range(FFC):
                    nc.tensor.matmul(ps2[:],
                                     h1gT[f][:, ql],
                                     w2_t[:, f * D:(f + 1) * D],
                                     start=(f == 0),
                                     stop=(nobias and f == FFC - 1))
                if not nobias:
                    nc.tensor.matmul(ps2[:], ones1b[:], brow_t[:, 2 * D:3 * D],
                                     start=False, stop=True)
                o = fpool.tile([128, D], F32, tag="fout", name="fout", bufs=2)
                nc.vector.tensor_add(o[:], ps2[:], x2_t[q][:])
                nc.sync.dma_start(out[ql, :], o[:])

            for qh in range(2):
                hsl = slice(CH * qh, CH * (qh + 1))
                for f in range(FFC):
                    ps = psF.tile([128, CH], F32, tag="f1", name="f1")
                    for k in range(KC):
                        nc.tensor.matmul(
                            ps[:],
                            w1_t[:, k * FF + 128 * f:k * FF + 128 * (f + 1)],
                            xs2T[k][:, hsl],
                            start=(k == 0), stop=(k == KC - 1))
                    nc.scalar.activation(h1gT[f][:, hsl], ps[:],
                                         AF.Gelu, bias=bf1_c(f))
                ffn2(2 * qh)
                ffn2(2 * qh + 1)
            dump("d_h1", h1gT[0][:])

    nc.compile()
    return nc


def _get_nc(W, uniform, nobias):
    key = (W, uniform, nobias)
    if key not in _BUILD_CACHE:
        _BUILD_CACHE[key] = _build(W, uniform, nobias)
    return _BUILD_CACHE[key]


# ---------------------------------------------------------------------------
# host side
# ---------------------------------------------------------------------------

def _bf16(a):
    return np.asarray(a, np.float32).astype(ml_dtypes.bfloat16)


def kernel(x, mask, wq, wk, wv, wo, bo, g1, b1, g2, b2, w1, bf1, w2, bf2,
           decay_logit):
    x = np.asarray(x, np.float32)
    mask = np.asarray(mask)
    g1 = np.asarray(g1, np.float64)
    b1 = np.asarray(b1, np.float64)
    g2 = np.asarray(g2, np.float64)
    b2 = np.asarray(b2, np.float64)

    d64 = 1.0 / (1.0 + np.exp(-np.asarray(decay_logit, np.float64)))
    d64 = np.clip(d64, 1e-8, None)
    dmax = float(d64.max())
    # Band width: the attention kernel truncates at |i-j| > ~W.  The omitted
    # mass relative to the normalizer z is ~ d^(W+1)/(1+d); keep it under
    # 1e-5, well below the bf16 matmul noise (~1e-3).
    if dmax >= 1.0 - 1e-12:
        W = 512  # decay ~1: widest supported band
    else:
        need = math.log(1e-5 * (1.0 + dmax)) / math.log(dmax)
        W = max(128, 128 * math.ceil(need / 128))
        W = min(W, 512)
    nW = W // 128
    L = TPC + 2 * W
    NT = L // 128
    uniform = bool(np.all(d64 == d64[0]))
    nobias = bool(np.all(b1 == 0) and np.all(np.asarray(bo) == 0)
                  and np.all(np.asarray(bf2) == 0))
    NHM = 1 if uniform else H
    NQP = 1 if uniform else KC

    nc = _get_nc(W, uniform, nobias)

    # folded weights (bf16)
    wq_f = _bf16(g1[:, None] * np.asarray(wq, np.float64))
    wk_f = _bf16(g1[:, None] * np.asarray(wk, np.float64))
    wv_f = _bf16(g1[:, None] * np.asarray(wv, np.float64))
    wo_f = _bf16(wo)
    w1_f = _bf16(g2[:, None] * np.asarray(w1, np.float64))
    w2_f = _bf16(w2)
    bq_h = (b1 @ np.asarray(wq, np.float64)).astype(np.float32)
    bk_h = (b1 @ np.asarray(wk, np.float64)).astype(np.float32)
    bv_h = _bf16(b1 @ np.asarray(wv, np.float64))
    bo_h = _bf16(bo)
    bf1_h = (b2 @ np.asarray(w1, np.float64) +
             np.asarray(bf1, np.float64)).astype(np.float32)
    bf2_h = _bf16(bf2)

    def pack_kblocks(w, blk, nb):
        # [nb*128, cols] -> [128, nb*cols] with block k at cols [k*cols, ...)
        return np.ascontiguousarray(
            w.reshape(nb, 128, blk).transpose(1, 0, 2).reshape(128, nb * blk))

    wa_h = np.concatenate([
        pack_kblocks(wk_f, 512, 4).reshape(128, -1),
        pack_kblocks(wq_f, 512, 4).reshape(128, -1),
        np.eye(128, dtype=ml_dtypes.bfloat16)], axis=1)
    wb_h = np.concatenate([
        pack_kblocks(wv_f, 512, 4),
        pack_kblocks(wo_f, 512, 4)], axis=1)
    w1_h = pack_kblocks(w1_f, FF, 4)
    w2_h = pack_kblocks(w2_f, D, 16)
    brow_h = np.concatenate(
        [bv_h, bo_h, bf2_h]).reshape(1, 3 * D).astype(ml_dtypes.bfloat16)

    # decay-derived tables
    ld = np.log(d64)                                    # (H,)
    di = np.arange(CH, dtype=np.float64)
    qposb = np.zeros((NQP, 128, CH), np.float32)
    qnegb = np.zeros((NQP, 128, CH), np.float32)
    for c in range(NQP):
        for hh in range(2):
            h = 2 * c + hh
            qposb[c, 64 * hh:64 * hh + 64, :] = np.exp(ld[h] * di)[None, :]
            qnegb[c, 64 * hh:64 * hh + 64, :] = np.exp(-ld[h] * di)[None, :]
    if uniform:
        qposb[:, 64:, :] = qposb[:, :64, :]
        qnegb[:, 64:, :] = qnegb[:, :64, :]
    # diagonal-zone mask: m[dk, di] = d^{|di-dk| - di}, dk in [0,256)
    dk = np.arange(CH, dtype=np.float64)
    dmask = np.zeros((NHM, 2, 128, CH), np.float32)
    for h in range(NHM):
        m = np.exp(ld[h] * (np.abs(di[None, :] - dk[:, None]) - di[None, :]))
        dmask[h, 0] = m[:128, :]
        dmask[h, 1] = m[128:, :]
    cb_h = np.concatenate([
        qposb.transpose(1, 0, 2).reshape(128, NQP * CH),
        qnegb.transpose(1, 0, 2).reshape(128, NQP * CH),
        dmask.transpose(2, 0, 1, 3).reshape(128, NHM * 2 * CH)], axis=1)
    cb_h = np.ascontiguousarray(cb_h, np.float32)

    in_maps = []
    for core in range(NCORES):
        b = core // SH
        t0 = (core % SH) * TPC
        lo, hi = t0 - W, t0 + TPC + W
        xs = np.zeros((L, D), np.float32)
        pad = np.zeros((L,), np.float64)
        glo, ghi = max(lo, 0), min(hi, T)
        xs[glo - lo:ghi - lo] = x[b, glo:ghi]
        pad[glo - lo:ghi - lo] = (~mask[b, glo:ghi]).astype(np.float64)
        xbig = np.ascontiguousarray(
            xs.reshape(NT, 128, D).transpose(1, 0, 2).reshape(128, NT * D))

        vneg = np.zeros((H, L), np.float32)
        vpos = np.zeros((H, L), np.float32)
        vpln = np.zeros((H, L), np.float32)
        for h in range(H):
            vn = np.zeros(L)
            vp = np.zeros(L)
            for qi in range(NQC):
                s = W + qi * CH
                jj = np.arange(s - W, s)
                vn[jj] = np.exp(ld[h] * (s - jj))
                jj = np.arange(s + CH, s + CH + W)
                vp[jj] = np.exp(ld[h] * (CH + jj - (s + CH)))
            vneg[h] = (vn * pad).astype(np.float32)
            vpos[h] = (vp * pad).astype(np.float32)
            vpln[h] = pad.astype(np.float32)

        def _vlayout(a):
            return np.ascontiguousarray(
                a.reshape(H, L // 128, 128).transpose(2, 1, 0)
                 .reshape(128, (L // 128) * H))
        vneg, vpos, vpln = _vlayout(vneg), _vlayout(vpos), _vlayout(vpln)

        cs_h = np.concatenate([
            bq_h.reshape(4, 128).T, bk_h.reshape(4, 128).T,
            bf1_h.reshape(16, 128).T,
            vneg, vpos, vpln], axis=1)
        cs_h = np.ascontiguousarray(cs_h, np.float32)

        nh = NT // 2
        im = {
            "xh0": np.ascontiguousarray(xbig[:, :nh * D]),
            "xh1": np.ascontiguousarray(xbig[:, nh * D:]),
            "cs": cs_h, "cb": cb_h,
            "wa": wa_h, "wb": wb_h, "w1": w1_h, "w2": w2_h,
        }
        if not nobias:
            im["brow"] = brow_h
        in_maps.append(im)

    res = bass_utils.run_bass_kernel_spmd(nc, in_maps,
                                          core_ids=list(range(NCORES)))
    out = np.empty((B, T, D), np.float32)
    for core in range(NCORES):
        b = core // SH
        t0 = (core % SH) * TPC
        out[b, t0:t0 + TPC] = res.results[core]["out"]

    # Degenerate-mask patch: a query whose entire +-W neighbourhood is
    # masked has z ~ 0 on the device; the reference output is then
    # dominated by out-of-band keys that the banded kernel truncates.
    # Recompute those rows exactly on the host (never triggers for an
    # all-False mask).
    if mask.any():
        idx = np.arange(T)
        for b_ in range(B):
            keep = ~np.asarray(mask[b_])
            if keep.all():
                continue
            kpos = idx[keep]
            if len(kpos) == 0:
                dist = np.full(T, T)
            else:
                ins = np.searchsorted(kpos, idx)
                left = np.where(ins > 0, idx - kpos[np.clip(ins - 1, 0,
                                                            len(kpos) - 1)], T)
                right = np.where(ins < len(kpos),
                                 kpos[np.clip(ins, 0, len(kpos) - 1)] - idx, T)
                dist = np.minimum(left, right)
            need = dist > (W - 32)
            if need.any():
                out[b_, need] = _exact_rows(
                    np.asarray(x[b_], np.float64), keep.astype(np.float64),
                    np.where(need)[0], wq, wk, wv, wo, bo, g1, b1, g2, b2,
                    w1, bf1, w2, bf2, d64)
    return out


def _exact_rows(xb, pad, rows, wq, wk, wv, wo, bo, g1, b1, g2, b2,
                w1, bf1, w2, bf2, d64):
    """Reference math (float64) for the given query rows of one batch."""
    import scipy.special as _sp

    def ln(z, g, b):
        mu = z.mean(-1, keepdims=True)
        var = ((z - mu) ** 2).mean(-1, keepdims=True)
        return (z - mu) / np.sqrt(var + 1e-5) * g + b

    wq = np.asarray(wq, np.float64); wk = np.asarray(wk, np.float64)
    wv = np.asarray(wv, np.float64); wo = np.asarray(wo, np.float64)
    w1 = np.asarray(w1, np.float64); w2 = np.asarray(w2, np.float64)
    xn = ln(xb, np.asarray(g1, np.float64), np.asarray(b1, np.float64))

    def feat(z):
        return np.where(z > 0, z + 1.0, np.exp(np.minimum(z, 0.0)))

    Q = feat((xn[rows] @ wq).reshape(len(rows), H, HD))       # (R,H,hd)
    K = feat((xn @ wk).reshape(T, H, HD)) * pad[:, None, None]
    V = (xn @ wv).reshape(T, H, HD) * pad[:, None, None]
    dist = np.abs(rows[:, None] - np.arange(T)[None, :]).astype(np.float64)
    M = np.exp(dist[None] * np.log(d64)[:, None, None])       # (H,R,T)
    A = np.einsum('rhd,jhd->hrj', Q, K) * M
    z = np.clip(A.sum(-1, keepdims=True), 1e-6, None)
    o = np.einsum('hrj,jhd->rhd', A / z, V).reshape(len(rows), D)
    o = o @ wo + np.asarray(bo, np.float64)
    x2 = xb[rows] + o
    xn2 = ln(x2, np.asarray(g2, np.float64), np.asarray(b2, np.float64))
    h1 = xn2 @ w1 + np.asarray(bf1, np.float64)
    gl = 0.5 * h1 * (1.0 + _sp.erf(h1 / np.sqrt(2.0)))
    ffn = gl @ w2 + np.asarray(bf2, np.float64)
    return (x2 + ffn).astype(np.float32)
